# revision 72
# baseline (speedup 1.0000x reference)
"""AttnBlock (GroupNorm -> single-head attention -> proj -> residual) on 8
Trainium2 NeuronCores.

Sharding: core = (b, s); b = core // 4 selects the batch element, s = core % 4
selects a 2048-wide query slice of N=8192. Each core receives x[b] rolled by
-2048*s along N so its queries are always columns 0..2047 (keys become a
permutation of N, which softmax/attention are invariant to). This keeps one
SPMD program with static addressing and no collectives.

Algebra (GroupNorm folded on host: h = s1*x + s2 per channel):
  scores  s = k^T q = x^T M x + (m^T x broadcast over keys), M = wk_f^T wq_f
          The key-side bias is constant per query so it cancels in softmax;
          only the query-side bias m = wk_f^T bq_f survives.
  attn    h2 = V e = wv_f (x e) + const, so with P = wp wv_f the projection
          is o = P (x e)/den + const; both constants fold into the residual.
  This removes the k and v pipelines entirely: the kernel computes only
  z = M x + m (a q-sized tensor), uses x itself as the score lhsT, and a
  host-transposed copy of x (DMA'd) as the lhsT of the (x e) accumulation.

Engine balance: softmax exp() is the throughput bottleneck (the scalar
engine owns the exp table), so 6 of the 32 exp tiles per chunk run on the
Vector engine instead as ((sD*(y+a))^2+b)^8 with fp16 intermediates (the
tensor_scalar steps hit the DVE 4x perf mode; scalar_tensor_tensor would
get none); the final fp8 squaring runs on the otherwise idle GpSimd(Pool)
engine (it cannot touch PSUM or run TensorScalarPtr). The Vector tiles'
(x e)/den accumulation matmuls are deferred ~12 slots because their e-tile
takes ~6us to emerge from the DVE+Pool pipeline -- the in-order PE must
never reach a matmul whose e-tile isn't ready, as that starves the scalar
engine too. Each chunk's den/normalize/projection tail is emitted during
the next chunk's early slots and shares the single PSUM rotation (3 bufs x
2 banks + 2 banks of x*e accumulator = all 8 banks). A -2 bias keeps
exp() inside fp8e4m3 range and cancels in the normalizer.
"""

import ml_dtypes
import numpy as np

import bass_rust
import concourse.bass as bass
import concourse.tile as tile
from concourse import mybir
from concourse.bass_utils import run_bass_kernel_spmd

B, C, N = 2, 256, 8192
NCORES = 8
NSLICE = 4          # query slices per batch element
MQ = N // NSLICE    # 2048 queries per core
CHUNK = 512         # queries processed per attention pass
JT = N // 128       # 64 key tiles
EPS = 1e-5
SCALE = C ** -0.5   # 0.0625

F32 = mybir.dt.float32
F16 = mybir.dt.float16
BF16 = mybir.dt.bfloat16
FP8 = mybir.dt.float8e4
BF16_NP = ml_dtypes.bfloat16
FP8_NP = ml_dtypes.float8_e4m3
AF = mybir.ActivationFunctionType
ALU = mybir.AluOpType

# Vector-engine exp approximation: with y = (s*SCALE-2)/8 in [-1.5, 0.56]
# (s*SCALE spans +-7 sigma), e^(8y) ~ ((sD*(y+a))^2 + b)^8. The affine in y
# folds into the first tensor_scalar's immediates; only tensor_scalar /
# tensor_tensor ops are used (scalar_tensor_tensor gets no DVE perf modes).
# Contribution-weighted max error ~2.7%; end-to-end attention error ~8% on
# the poly tiles, comparable to the fp8 weight quantization already in use.
PA1 = 96.43778996219165     # (a - 0.25) / (SCALE/8)
PSD = 0.0056569268563881305  # sD * (SCALE/8)
PB2 = 0.46313871554700836   # b

# jj slots (of 32 per chunk) whose exp runs on the Vector engine. All PSUM
# score/den/tail tiles share one 3-deep rotation, so the PE only stalls if
# a consumer lags by 3 full allocations (~3.5us) -- enough slack for the
# Vector chain's queue latency. SQ2_DVE=99: every Vector tile keeps its 2nd
# squaring on DVE (keeps e-tile latency ~6us; Pool does only the final fp8
# squaring). Keeping the Vector engine under ~70% load is essential: its
# queue latency otherwise exceeds any PSUM-rotation slack and the in-order
# PE stalls, which starves the scalar engine too.
DVE_JJS = (2, 7, 12, 17, 22, 27)      # chunks 0-2
DVE_JJS_LAST = (2, 7, 12, 17, 22, 27)  # last chunk (kept identical;
                                       # early placements measured worse)
SQ2_DVE = 99        # all 2nd squarings on DVE (shorter e-tile latency)
ACC_DEFER = 9       # slots between an exp issue and its xe accumulation
DVE_DEFER = 16      # same for Vector tiles (their e-tile has ~6us latency)


# ---------------------------------------------------------------------------
# Workaround: this container's walrus build rejects any instruction carrying
# more than one semaphore wait ("Too many sync wait commands"). Two pieces:
# (1) the Tile exit drain gets its waits split across per-proc sync nops;
# (2) a post-pass hoists excess waits from scheduled instructions onto
#     same-engine NoOps inserted immediately before them (same engine +
#     program order => identical blocking semantics).
def _drain_and_barrier_split(self, tick_clock, wait_clock):
    gc = tick_clock.global_clock
    vals = list(gc)
    n = len(vals)
    for i, v in enumerate(vals):
        if v == 0:
            continue
        vec = [0] * n
        vec[i] = v
        nop = self.nc.sync.nop(nofuse=True, hint=f"drain_split_{i}")
        wait_clock.add_sem_waits(
            nop.ins, bass_rust.ScopedClock({None: bass_rust.VectorClock(vec)})
        )
    self.nc.sync.drain()
    self.nc.all_engine_barrier()
    assert self.sems is not None
    popped = self.nc._tile_sem_poison_stack.pop()
    assert popped is self._sem_poison
    self.nc.clear_and_free_semaphores(list(self.sems.allocated().values()))
    self.nc.all_engine_barrier()


tile.TileContext._drain_and_barrier = _drain_and_barrier_split


def _split_excess_waits(nc, max_waits=1):
    for f in nc.m.functions:
        for blk in f.blocks:
            il = blk.instructions
            out = []
            changed = False
            for inst in il:
                si = getattr(inst, "sync_info", None)
                waits = list(si.on_wait) if si is not None and si.on_wait else []
                if len(waits) > max_waits:
                    for k, w in enumerate(waits[:-max_waits]):
                        nop = bass_rust.InstNoOp(
                            name=f"{inst.name}-wsplit{k}", ins=[], outs=[])
                        nop.engine = inst.engine
                        nop.sync_info = bass_rust.SyncInfo(
                            on_wait=[w], on_update=[])
                        out.append(nop)
                    si.on_wait = waits[-max_waits:]
                    changed = True
                out.append(inst)
            if changed:
                il[:] = out
# ---------------------------------------------------------------------------


def build_program() -> bass.Bass:
    nc = bass.Bass("TRN2", target_bir_lowering=False, debug=False)

    x_d = nc.dram_tensor("x", [128, 2, N], FP8, kind="ExternalInput").ap()
    xT_d = nc.dram_tensor("xT", [128, JT // 4, 4, 256], FP8,
                          kind="ExternalInput").ap()
    xr_d = nc.dram_tensor("xres", [128, 2, MQ], F32, kind="ExternalInput").ap()
    wm_d = nc.dram_tensor("wm", [128, 2, 2, 128], FP8, kind="ExternalInput").ap()
    wp_d = nc.dram_tensor("wp", [128, 2, 2, 128], BF16, kind="ExternalInput").ap()
    bm_d = nc.dram_tensor("bm", [128, 2], F32, kind="ExternalInput").ap()
    out_d = nc.dram_tensor("out", [128, 2, MQ], F32, kind="ExternalOutput").ap()

    with tile.TileContext(nc) as tc:
        with (
            tc.tile_pool(name="consts", bufs=1) as consts,
            tc.tile_pool(name="hsb", bufs=1) as hpool,
            tc.tile_pool(name="stats", bufs=1) as stats,
            tc.tile_pool(name="pp", bufs=3, space="PSUM") as pp,
            tc.tile_pool(name="xep", bufs=1, space="PSUM") as xep,
        ):
            # ---- constants -------------------------------------------------
            wm_sb = consts.tile([128, 2, 2, 128], FP8)
            wp_sb = consts.tile([128, 2, 2, 128], BF16)
            bm_sb = consts.tile([128, 2], F32)
            ones_sb = consts.tile([128, 2, 16], FP8)
            onesf_sb = consts.tile([1, 128], BF16)
            nb2_sb = consts.tile([128, 1], F32)
            for dst, srcap in [(wm_sb, wm_d), (bm_sb, bm_d)]:
                nc.sync.dma_start(out=dst, in_=srcap)
            nc.vector.memset(ones_sb, 1.0)
            nc.vector.memset(onesf_sb, 1.0)
            nc.vector.memset(nb2_sb, -2.0)

            xt = hpool.tile([128, 2, N], FP8)
            xTt = hpool.tile([128, JT // 4, 4, 256], FP8)
            # DMA in consumption order: z reads both ci planes of the first
            # query columns; score lhsT sweeps xt keys at ~1 slice / 4 slots;
            # the xT slices feed the deferred xe accumulations slightly later
            def dma_xt(qd):
                qsl = slice(qd * (N // 8), (qd + 1) * (N // 8))
                for ci in range(2):
                    nc.sync.dma_start(out=xt[:, ci, qsl], in_=x_d[:, ci, qsl])

            def dma_xT(s4):
                jsl = slice(s4 * 4, s4 * 4 + 4)
                nc.sync.dma_start(out=xTt[:, jsl], in_=xT_d[:, jsl])

            for ci in range(2):   # z np2-0 needs only cols 0:1024
                nc.sync.dma_start(out=xt[:, ci, 0:1024], in_=x_d[:, ci, 0:1024])
            for ci in range(2):
                nc.sync.dma_start(out=xt[:, ci, 1024:2048],
                                  in_=x_d[:, ci, 1024:2048])
            dma_xT(0); dma_xt(2), dma_xt(3); dma_xT(1)
            dma_xt(4); dma_xt(5); dma_xT(2); dma_xt(6)
            dma_xT(3); dma_xt(7)
            nc.sync.dma_start(out=wp_sb, in_=wp_d)

            with (
                tc.tile_pool(name="kqv", bufs=1) as kqv,
                tc.tile_pool(name="esb", bufs=44) as epool,
                tc.tile_pool(name="fsb", bufs=2) as fpool,
                tc.tile_pool(name="tsb", bufs=2) as tpool,
                tc.tile_pool(name="osb", bufs=3) as opool,
            ):
                zt = kqv.tile([128, 2, MQ], FP8)
                DR = mybir.MatmulPerfMode.DoubleRow

                # ---- phase B: z = M x + m (q-sized only; k/v eliminated) ---
                for np2 in range(MQ // 1024):
                    for ot in range(2):
                        ps2 = pp.tile([128, 2, 512], F32, tag="ps",
                                      name=f"psz_{np2}_{ot}")
                        for r in range(2):
                            sl = slice(np2 * 1024 + r * 512,
                                       np2 * 1024 + r * 512 + 512)
                            nc.tensor.matmul(ps2[:, r, :],
                                             lhsT=wm_sb[:, :, ot, :],
                                             rhs=xt[:, :, sl], perf_mode=DR,
                                             start=True, stop=True)
                        osl = slice(np2 * 1024, np2 * 1024 + 1024)
                        if ot == 0 and np2 == 0:
                            # only the startup-critical drain uses the scalar
                            # engine; the rest stay off the exp stream
                            nc.scalar.activation(out=zt[:, 0, osl], in_=ps2,
                                                 func=AF.Identity,
                                                 bias=bm_sb[:, 0:1])
                        else:
                            nc.vector.tensor_scalar_add(
                                out=zt[:, ot, osl], in0=ps2,
                                scalar1=bm_sb[:, ot:ot + 1])

                # ---- phase C: attention + projection per 512-query chunk ---
                # Each chunk's denominator + normalize + projection tail is
                # emitted during the NEXT chunk's early slots: the den
                # accumulation (32 PE matmuls over the kept e-tiles) and the
                # broadcast/projection PSUM tiles ride the shared rotation,
                # so the tail never sits on the scalar-engine critical path.
                def make_tail(mc, ets, xe, xr, halves=False):
                    msl = slice(mc * CHUNK, mc * CHUNK + CHUNK)
                    den_t = [None]
                    rd_row = stats.tile([1, CHUNK], BF16, tag="rdrow",
                                        name=f"rd_{mc}")
                    rdb = opool.tile([128, CHUNK], F32, tag="rdb",
                                     name=f"rdb_{mc}")
                    xen = tpool.tile([128, 2, CHUNK], BF16, name=f"xen_{mc}")

                    def den_group(g):
                        if g == 0:
                            den_t[0] = pp.tile([1, CHUNK], F32, tag="ps",
                                               name=f"den_{mc}")
                        for i in range(8 * g, 8 * g + 8):
                            nc.tensor.matmul(den_t[0],
                                             lhsT=ones_sb[:, :, 0:1],
                                             rhs=ets[i], perf_mode=DR,
                                             start=(i == 0), stop=(i == 31))
                        if g == 3 and not halves:
                            with nc.allow_low_precision(
                                    reason="softmax denom bf16"):
                                nc.vector.reciprocal(rd_row, den_t[0])

                    def bc_stage():
                        ps_bc = pp.tile([128, CHUNK], F32, tag="ps",
                                        name=f"psbc_{mc}")
                        nc.tensor.matmul(ps_bc, lhsT=onesf_sb, rhs=rd_row,
                                         start=True, stop=True)
                        nc.vector.tensor_copy(out=rdb, in_=ps_bc)

                    def norm():
                        for ci in range(2):
                            nc.vector.tensor_tensor(out=xen[:, ci, :],
                                                    in0=xe[:, ci, :], in1=rdb,
                                                    op=ALU.mult)

                    def proj_out():
                        for ot in range(2):
                            ps_o = pp.tile([128, CHUNK], F32, tag="ps",
                                           name=f"pso_{mc}_{ot}")
                            for ci in range(2):
                                nc.tensor.matmul(ps_o,
                                                 lhsT=wp_sb[:, ci, ot, :],
                                                 rhs=xen[:, ci, :],
                                                 start=(ci == 0),
                                                 stop=(ci == 1))
                            o_sb = opool.tile([128, CHUNK], F32, tag="o_sb")
                            nc.vector.tensor_tensor(out=o_sb, in0=ps_o,
                                                    in1=xr[:, ot, :],
                                                    op=ALU.add)
                            nc.sync.dma_start(out=out_d[:, ot, msl], in_=o_sb)

                    def tail_half(h):
                        # final chunk: process 256-query halves so the
                        # serial recip->bc->stage->normalize->project->DMA
                        # chain pipelines across engines
                        hs = slice(h * 256, h * 256 + 256)
                        with nc.allow_low_precision(
                                reason="softmax denom bf16"):
                            nc.vector.reciprocal(rd_row[:, hs],
                                                 den_t[0][:, hs])
                        ps_bc = pp.tile([128, 256], F32, tag="ps",
                                        name=f"psbc_{mc}_{h}")
                        nc.tensor.matmul(ps_bc, lhsT=onesf_sb,
                                         rhs=rd_row[:, hs],
                                         start=True, stop=True)
                        rdb_h = opool.tile([128, 256], F32, tag="rdb")
                        nc.vector.tensor_copy(out=rdb_h, in_=ps_bc)
                        for ci in range(2):
                            nc.vector.tensor_tensor(out=xen[:, ci, hs],
                                                    in0=xe[:, ci, hs],
                                                    in1=rdb_h, op=ALU.mult)
                        for ot in range(2):
                            ps_o = pp.tile([128, 256], F32, tag="ps")
                            for ci in range(2):
                                nc.tensor.matmul(ps_o,
                                                 lhsT=wp_sb[:, ci, ot, :],
                                                 rhs=xen[:, ci, hs],
                                                 start=(ci == 0),
                                                 stop=(ci == 1))
                            o_sb = opool.tile([128, 256], F32, tag="o_sb")
                            nc.vector.tensor_tensor(out=o_sb, in0=ps_o,
                                                    in1=xr[:, ot, hs],
                                                    op=ALU.add)
                            osl = slice(mc * CHUNK + h * 256,
                                        mc * CHUNK + h * 256 + 256)
                            nc.sync.dma_start(out=out_d[:, ot, osl],
                                              in_=o_sb)

                    if halves:
                        return [(2, lambda: den_group(0)),
                                (3, lambda: den_group(1)),
                                (4, lambda: den_group(2)),
                                (5, lambda: den_group(3)),
                                (6, lambda: tail_half(0)),
                                (7, lambda: tail_half(1))]
                    return [(2, lambda: den_group(0)), (3, lambda: den_group(1)),
                            (4, lambda: den_group(2)), (5, lambda: den_group(3)),
                            (6, bc_stage), (7, norm), (8, proj_out)]

                prev_tail = []
                for mc in range(MQ // CHUNK):
                    dve_jjs = (DVE_JJS_LAST if mc == MQ // CHUNK - 1
                               else DVE_JJS)
                    msl = slice(mc * CHUNK, mc * CHUNK + CHUNK)
                    xe = xep.tile([128, 2, CHUNK], F32, tag="xe",
                                  name=f"xe_{mc}")
                    xr = opool.tile([128, 2, CHUNK], F32, tag="xr")
                    nc.sync.dma_start(out=xr, in_=xr_d[:, :, msl])

                    ets = []
                    pending = []   # (ready_slot, jj, et) awaiting xe matmuls
                    n_acc = 0

                    def acc_xe(jj, et):
                        nonlocal n_acc
                        first, last = n_acc == 0, n_acc == JT // 2 - 1
                        n_acc += 1
                        for ci in range(2):
                            nc.tensor.matmul(
                                xe[:, ci, :],
                                lhsT=xTt[:, jj // 2,
                                         2 * (jj % 2):2 * (jj % 2) + 2,
                                         ci * 128:ci * 128 + 128],
                                rhs=et, perf_mode=DR,
                                start=first, stop=last)

                    for jj in range(JT // 2):
                        for off, fn in prev_tail:
                            if off == jj:
                                fn()
                        et = epool.tile([128, 2, CHUNK], FP8)
                        ets.append(et)
                        is_dve = jj in dve_jjs
                        ps2 = pp.tile([128, 2, CHUNK], F32, tag="ps")
                        for r in range(2):
                            j = 2 * jj + r
                            jsl = slice(j * 128, j * 128 + 128)
                            nc.tensor.matmul(ps2[:, r, :],
                                             lhsT=xt[:, :, jsl],
                                             rhs=zt[:, :, msl], perf_mode=DR,
                                             start=True, stop=True)
                        if is_dve:
                            # Vector-engine exp: ((sD*(y+a))^2 + b)^8 with
                            # fp16 intermediates (tensor_scalar runs 4x in
                            # SBUF); the last fp8 squaring on the Pool engine
                            t1 = fpool.tile([128, 2, CHUNK], F16, tag="t1")
                            nc.vector.tensor_scalar(
                                out=t1, in0=ps2, scalar1=PA1, scalar2=PSD,
                                op0=ALU.add, op1=ALU.mult)
                            nc.vector.tensor_tensor(
                                out=t1, in0=t1, in1=t1, op=ALU.mult)
                            nc.vector.tensor_scalar(
                                out=t1, in0=t1, scalar1=PB2, scalar2=1.0,
                                op0=ALU.add, op1=ALU.mult)
                            nc.vector.tensor_tensor(
                                out=t1, in0=t1, in1=t1, op=ALU.mult)
                            if dve_jjs.index(jj) >= len(dve_jjs) - SQ2_DVE:
                                nc.vector.tensor_tensor(
                                    out=t1, in0=t1, in1=t1, op=ALU.mult)
                            else:
                                t2 = fpool.tile([128, 2, CHUNK], F16,
                                                tag="t2")
                                nc.gpsimd.tensor_tensor(
                                    out=t2, in0=t1, in1=t1, op=ALU.mult)
                                t1 = t2
                            nc.gpsimd.tensor_tensor(
                                out=et, in0=t1, in1=t1, op=ALU.mult)
                        else:
                            # -2 bias keeps exp() inside fp8e4m3 range; it
                            # cancels in the softmax normalizer.
                            nc.scalar.activation(out=et, in_=ps2, func=AF.Exp,
                                                 scale=SCALE, bias=nb2_sb)
                        pending.append(
                            (jj + (DVE_DEFER if is_dve else ACC_DEFER), jj, et))
                        pending.sort()
                        while pending and pending[0][0] <= jj:
                            acc_xe(*pending.pop(0)[1:])
                    for _, jj2, et2 in pending:
                        acc_xe(jj2, et2)
                    prev_tail = make_tail(mc, ets, xe, xr)
                for off, fn in prev_tail:
                    fn()
    _split_excess_waits(nc)
    return nc


_NC_CACHE = None


def _get_program():
    global _NC_CACHE
    if _NC_CACHE is None:
        _NC_CACHE = build_program()
    return _NC_CACHE


def _prep_batch(inputs, b, x):
    """Fold GroupNorm (stats computed here on the host) into the weights for
    batch element b: h = s1*x + s2 per channel, so W @ h = (W*diag(s1)) @ x
    + W @ s2. Then fuse across the attention algebra:
      M = wk_f^T wq_f, m = wk_f^T bq_f (k-side bias cancels in softmax),
      P = wp wv_f, residual offset bp_eff = bp + wp (bv + wv s2)."""
    f32 = np.float32
    wq = np.asarray(inputs["wq"], f32)
    wk = np.asarray(inputs["wk"], f32)
    wv = np.asarray(inputs["wv"], f32)
    wp = np.asarray(inputs["wp"], f32)
    bv = np.asarray(inputs["bv"], f32)
    bp = np.asarray(inputs["bp"], f32)
    gw = np.asarray(inputs["gn_weight"], f32)
    gb = np.asarray(inputs["gn_bias"], f32)

    g = x[b].reshape(32, 8 * N)
    mean = g.mean(axis=1)
    var = g.var(axis=1)
    rstd = 1.0 / np.sqrt(var + EPS)
    s1 = np.repeat(rstd, 8) * gw                       # [C]
    s2 = gb - np.repeat(mean * rstd, 8) * gw           # [C]

    wq_f = wq * s1[None, :]
    wk_f = wk * s1[None, :]
    wv_f = wv * s1[None, :]
    bq_f = np.asarray(inputs["bq"], f32) + wq @ s2
    M = wk_f.T @ wq_f                                  # [C, C]
    m = wk_f.T @ bq_f                                  # [C]
    P = wp @ wv_f                                      # [C, C]
    bp_f = bp + wp @ (bv + wv @ s2)

    def wT_pack(w, dt):  # [o, c] -> [p, ci, ot, o_local] of w.T
        return np.ascontiguousarray(
            w.T.reshape(2, 128, 2, 128).transpose(1, 0, 2, 3)
        ).astype(dt)

    return {
        "wm": wT_pack(M, FP8_NP),
        "wp": wT_pack(P, BF16_NP),
        "bm": np.ascontiguousarray(m.reshape(2, 128).T),
    }, bp_f


def kernel(**inputs) -> np.ndarray:
    x = np.asarray(inputs["x"], np.float32)  # [B, C, N]

    in_maps = []
    for b in range(B):
        shared_b, bp_f = _prep_batch(inputs, b, x)
        xb8 = x[b].astype(FP8_NP)  # convert once, roll per slice
        for s in range(NSLICE):
            xr8 = np.roll(xb8, -MQ * s, axis=1)  # queries at columns 0..MQ-1
            x_in = np.ascontiguousarray(
                xr8.reshape(2, 128, N).transpose(1, 0, 2))
            # transposed copy for the (x e) accumulation lhsT:
            # xT[p, jq, r, c] = x[c, 128*(4*jq+r)+p]
            xT_in = np.ascontiguousarray(
                xr8.T.reshape(JT // 4, 4, 128, C).transpose(2, 0, 1, 3))
            xres = np.ascontiguousarray(
                (x[b][:, MQ * s:MQ * (s + 1)] + bp_f[:, None])
                .reshape(2, 128, MQ).transpose(1, 0, 2))
            in_maps.append({"x": x_in, "xT": xT_in, "xres": xres, **shared_b})

    nc = _get_program()
    res = run_bass_kernel_spmd(nc, in_maps, core_ids=list(range(NCORES)))

    out = np.empty((B, C, N), np.float32)
    for core in range(NCORES):
        b, s = divmod(core, NSLICE)
        r = res.results[core]["out"]  # [128, 2, MQ]
        out[b][:, MQ * s:MQ * (s + 1)] = r.transpose(1, 0, 2).reshape(C, MQ)
    return out
